# revision 24
# baseline (speedup 1.0000x reference)
"""Trainium2 Bass kernel for a 2-layer multi-head GAT (nn_MultiHeadGATLayer).

Architecture recap (hardcoded, matches the reference):
  N=16384 nodes, D=512 feats, E=540672 edges (32 random in-edges/node + self loop),
  layer 1: 8 heads x 64 dims with per-head attention + elu, concat;
  layer 2: single 512-dim GAT head over the concat + elu; residual with input.

Distribution: nodes are sharded across 8 NeuronCores (destination sharding).
Each core computes z = x @ W for its node shard, the shards are AllGathered
into a per-core z-table in HBM, and each core then runs the edge phase for its
own destination nodes.

Performance structure (v2):
  - Gathers are SWDGE prepare_only + trigger_dma on 2 alternating queues so
    descriptor generation (GPSIMD) decouples from the DMA transfer; gather
    buffers are 6-deep so transfers pipeline back-to-back.
  - Layer-2 z rows are computed per-tile inside the layer-1 edge loop and
    AllGathered in 4 chunks that overlap the layer-1 edge phase.
  - Layer-2 edge accumulate uses diag(exp) as the matmul stationary operand
    (one [P,KG,128] diag build per group) instead of per-slot DVE multiplies.
  - PSUM->SBUF z-row copies run on the Scalar engine.

Host side does layout only: degree-sorted node permutation, padded gather
index construction (int16, 16-partition wrap), weight reshapes/transposes.
"""
import os
import sys

sys.path.insert(0, "/opt/trn_rl_repo")

import numpy as np
import ml_dtypes

import concourse.bacc as bacc
import concourse.mybir as mybir
from concourse.tile import TileContext
from concourse.bass_utils import run_bass_kernel_spmd
from concourse.library_config import mlp

F32 = mybir.dt.float32
BF16 = mybir.dt.bfloat16
I16 = mybir.dt.int16

B, S, D = 64, 256, 512
H, DO = 8, 64
ALPHA = 0.2
N = B * S
DEG = 32
E = N * DEG + N
NCORES = 8
P = 128
SHN = N // NCORES          # nodes per core (2048)
NT = SHN // P              # node tiles per core (16)
KG = 8                     # slots per gather call
ROWW = 640                 # z-table row width (512 z + 8 es + pad), bf16
DUMMY = N                  # dummy row index for padding slots
NEG = -3.0e38
NCH = 4                    # zloc2 AllGather chunks
TPC = NT // NCH            # tiles per chunk

_cache = {}


def _build_host(src, dst):
    """Host-side layout: permutation, per-core padded gather indices.

    ztab1 is laid out core-major (row = core*SHN + pos, the natural
    one-shot AllGather order).  ztab2 is laid out chunk-major
    (row = chunk*(NCORES*CHR) + core*CHR + pos%CHR, CHR = SHN//NCH) so each
    of the NCH chunked AllGathers writes a contiguous row range.
    """
    deg = np.bincount(dst, minlength=N)
    order = np.argsort(-deg, kind="stable")          # nodes by degree desc
    # deal round-robin so all cores see the same degree profile
    core_of = np.empty(N, np.int32)
    pos_of = np.empty(N, np.int32)
    for c in range(NCORES):
        nodes_c = order[c::NCORES]                   # 2048 nodes, deg-sorted desc
        core_of[nodes_c] = c
        pos_of[nodes_c] = np.arange(SHN)
    nodes = [order[c::NCORES] for c in range(NCORES)]
    core64 = core_of.astype(np.int64)
    pos64 = pos_of.astype(np.int64)
    tabpos1 = (core64 * SHN + pos64).astype(np.int32)
    CHR = SHN // NCH
    tabpos2 = ((pos64 // CHR) * (NCORES * CHR) + core64 * CHR
               + (pos64 % CHR)).astype(np.int32)

    # in-edges per node: sort edges by dst
    eorder = np.argsort(dst, kind="stable")
    src_s = src[eorder]
    cum = np.zeros(N + 1, np.int64)
    np.cumsum(deg, out=cum[1:])

    # common slot schedule: K_sched[j] = max over cores of tile max degree
    K_sched = np.zeros(NT, np.int64)
    for c in range(NCORES):
        dg = deg[nodes[c]]
        for j in range(NT):
            K_sched[j] = max(K_sched[j], dg[j * P:(j + 1) * P].max())
    K_sched = ((K_sched + KG - 1) // KG) * KG
    totK = int(K_sched.sum())

    # padded source table per core: [NT, K_j, P] slot-major within tile
    def build_idx(tabpos):
        idx_cores = []
        for c in range(NCORES):
            blocks = []
            nds = nodes[c]
            for j in range(NT):
                nj = nds[j * P:(j + 1) * P]
                Kj = int(K_sched[j])
                pad = np.full((P, Kj), DUMMY, np.int32)
                for i, n in enumerate(nj):
                    d0 = int(deg[n])
                    pad[i, :d0] = tabpos[src_s[cum[n]:cum[n] + d0]]
                blocks.append(pad.T.reshape(-1))     # slot-major: [Kj, P]
            flat = np.concatenate(blocks).astype(np.int32)   # [totK*P]
            # int16 wrap: idx i -> partition i%16, col i//16 ; replicate 8x
            assert flat.max() <= 32767
            w = flat.reshape(-1, 16).T               # [16, totK*P/16]
            idx_cores.append(np.tile(w, (8, 1)).astype(np.int16))
        return idx_cores

    return nodes, K_sched, totK, build_idx(tabpos1), build_idx(tabpos2)


def _build_program(K_sched, totK):
    nc = bacc.Bacc("TRN2", target_bir_lowering=False, debug=False,
                   num_devices=NCORES, num_swdge_queues=2)
    KT = [int(k) for k in K_sched]
    IDXW = totK * P // 16

    xT_in = nc.dram_tensor("xT", [D, SHN], F32, kind="ExternalInput")
    x_in = nc.dram_tensor("x", [SHN, D], F32, kind="ExternalInput")
    w1_in = nc.dram_tensor("w1", [D, D], F32, kind="ExternalInput")       # W1cat
    w1t_in = nc.dram_tensor("w1t", [D, D], F32, kind="ExternalInput")     # W1cat.T
    a1_in = nc.dram_tensor("a1", [D, 16], F32, kind="ExternalInput")      # [A1s | A1d]
    w2_in = nc.dram_tensor("w2", [D, D], F32, kind="ExternalInput")       # Wout
    w2t_in = nc.dram_tensor("w2t", [D, D], F32, kind="ExternalInput")     # Wout.T
    a2_in = nc.dram_tensor("a2", [D, 2], F32, kind="ExternalInput")       # [aout_s | aout_d]
    id_in = nc.dram_tensor("ident", [P, P], F32, kind="ExternalInput")
    idx1_in = nc.dram_tensor("idx1", [P, IDXW], I16, kind="ExternalInput")
    idx2_in = nc.dram_tensor("idx2", [P, IDXW], I16, kind="ExternalInput")
    out = nc.dram_tensor("out", [SHN, D], F32, kind="ExternalOutput")

    zloc1 = nc.dram_tensor("zloc1", [SHN, ROWW], BF16, kind="Internal")
    zloc2 = nc.dram_tensor("zloc2", [SHN, ROWW], BF16, kind="Internal")
    ztab1 = nc.dram_tensor("ztab1", [N + P, ROWW], BF16, kind="Internal",
                           addr_space="Shared")
    ztab2 = nc.dram_tensor("ztab2", [N + P, ROWW], BF16, kind="Internal",
                           addr_space="Shared")

    with TileContext(nc) as tc:
        with tc.tile_pool(name="const", bufs=1) as cpool, \
             tc.tile_pool(name="work", bufs=2) as wpool, \
             tc.tile_pool(name="gat", bufs=6) as gpool, \
             tc.tile_pool(name="wg", bufs=3) as wgpool, \
             tc.tile_pool(name="pz", bufs=2, space="PSUM") as pzpool, \
             tc.tile_pool(name="pe", bufs=2, space="PSUM") as pepool, \
             tc.tile_pool(name="pt", bufs=2, space="PSUM") as ptpool:

            nc.gpsimd.load_library(mlp)

            dma_sems = [nc.alloc_semaphore(f"gdma{q}") for q in range(2)]
            for s in dma_sems:
                nc.gpsimd.sem_clear(s)

            # In prep mode Tile drops the collective->gather RAW dep (it
            # should defer to the trigger but doesn't), so gate the in-order
            # GPSIMD stream: DMA-read a row the collective wrote, then have
            # GPSIMD consume it before the first prep of the edge phase.
            def gpsimd_gate(ztab, row_ids, tag):
                chk = cpool.tile([1, len(row_ids), 2], BF16, tag=f"chk{tag}")
                for j, r in enumerate(row_ids):
                    nc.sync.dma_start(chk[:, j, :], ztab[r:r + 1, 0:2])
                gate = cpool.tile([1, 2], BF16, tag=f"gate{tag}")
                for j in range(len(row_ids)):
                    nc.gpsimd.tensor_copy(gate[:], chk[:, j, :])

            # ---------- setup: constants, weights ----------
            identf = cpool.tile([P, P], F32)
            identb = cpool.tile([P, P], BF16)
            nc.sync.dma_start(identf[:], id_in[:])
            nc.vector.tensor_copy(identb[:], identf[:])

            idx1 = cpool.tile([P, IDXW], I16)
            idx2 = cpool.tile([P, IDXW], I16)
            nc.sync.dma_start(idx1[:], idx1_in[:])
            nc.sync.dma_start(idx2[:], idx2_in[:])

            # weights: rhs chunks [128, 512] bf16 (4 per layer) + va cols
            w1b, w2b, w1tf, w2tf = [], [], [], []
            for cidx in range(4):
                wf = wpool.tile([P, D], F32, tag="wload")
                nc.sync.dma_start(wf[:], w1_in[cidx * P:(cidx + 1) * P, :])
                wb = cpool.tile([P, D], BF16, tag=f"w1b{cidx}")
                nc.vector.tensor_copy(wb[:], wf[:])
                w1b.append(wb)
                wf2 = wpool.tile([P, D], F32, tag="wload")
                nc.sync.dma_start(wf2[:], w2_in[cidx * P:(cidx + 1) * P, :])
                wb2 = cpool.tile([P, D], BF16, tag=f"w2b{cidx}")
                nc.vector.tensor_copy(wb2[:], wf2[:])
                w2b.append(wb2)
                t1 = cpool.tile([P, D], F32, tag=f"w1t{cidx}")
                nc.sync.dma_start(t1[:], w1t_in[cidx * P:(cidx + 1) * P, :])
                w1tf.append(t1)
                t2 = cpool.tile([P, D], F32, tag=f"w2t{cidx}")
                nc.sync.dma_start(t2[:], w2t_in[cidx * P:(cidx + 1) * P, :])
                w2tf.append(t2)

            a1f = [cpool.tile([P, 16], F32, tag=f"a1f{c}", name=f"a1f{c}") for c in range(4)]
            a2f = [cpool.tile([P, 2], F32, tag=f"a2f{c}", name=f"a2f{c}") for c in range(4)]
            for cidx in range(4):
                nc.sync.dma_start(a1f[cidx][:], a1_in[cidx * P:(cidx + 1) * P, :])
                nc.sync.dma_start(a2f[cidx][:], a2_in[cidx * P:(cidx + 1) * P, :])

            # va1[d,16] = W1cat.T-chunks.T @ A1 blocks ; va2[d,2]
            va1 = []
            va2 = []
            for dchunk in range(4):
                pv = pzpool.tile([P, 16], F32, tag="pzA")
                for fc in range(4):
                    nc.tensor.matmul(
                        pv[:], w1tf[fc][:, dchunk * P:(dchunk + 1) * P],
                        a1f[fc][:],
                        start=(fc == 0), stop=(fc == 3))
                vb = cpool.tile([P, 16], BF16, tag=f"va1{dchunk}")
                nc.vector.tensor_copy(vb[:], pv[:])
                va1.append(vb)
                pv2 = pzpool.tile([P, 2], F32, tag="pzB", bufs=1)
                for fc in range(4):
                    nc.tensor.matmul(
                        pv2[:], w2tf[fc][:, dchunk * P:(dchunk + 1) * P],
                        a2f[fc][:],
                        start=(fc == 0), stop=(fc == 3))
                vb2 = cpool.tile([P, 2], BF16, tag=f"va2{dchunk}")
                nc.vector.tensor_copy(vb2[:], pv2[:])
                va2.append(vb2)

            # dummy row (padding target): zeros except es cols = NEG
            drow = cpool.tile([1, ROWW], BF16)
            nc.vector.memset(drow[:], 0.0)
            nc.vector.memset(drow[:, 512:520], NEG)
            for ztab in (ztab1, ztab2):
                nc.sync.dma_start(ztab[N:N + 1, :], drow[:])

            ed1 = cpool.tile([P, NT, H], F32)
            ed2 = cpool.tile([P, NT, 1], F32)
            hcTb = [cpool.tile([P, SHN], BF16, tag=f"hcT{c}", name=f"hcT{c}") for c in range(4)]

            # ---------- gather emitter: prep + trigger on alternating queues --
            qtog = [0]

            PREP = bool(int(os.environ.get("GAT_PREP", "1")))

            def emit_gather(g, ztab, idx, idx_off, nidx):
                if not PREP:
                    nc.gpsimd.dma_gather(
                        g[:], ztab[:], idx[:, idx_off:idx_off + nidx // 16],
                        nidx, nidx, ROWW)
                    return
                q = qtog[0]
                qtog[0] = 1 - q
                nc.gpsimd.dma_gather(
                    g[:], ztab[:], idx[:, idx_off:idx_off + nidx // 16],
                    nidx, nidx, ROWW,
                    prepare_only=True, sem=dma_sems[q], queue_num=q)
                nc.gpsimd.trigger_dma(count=None, queue_num=q)

            # ---------- z tile: z row + attn cols for one 128-node tile ------
            def z_tile(nt, lhs_blocks, wb, va, zloc, ed_store, ncols):
                pa = pzpool.tile([P, D], F32, tag="pzA")
                pb = pzpool.tile([P, 16], F32, tag="pzB", bufs=1)
                for cidx in range(4):
                    lb = lhs_blocks[cidx][:, nt * P:(nt + 1) * P]
                    nc.tensor.matmul(pa[:], lb, wb[cidx][:],
                                     start=(cidx == 0), stop=(cidx == 3))
                for cidx in range(4):
                    lb = lhs_blocks[cidx][:, nt * P:(nt + 1) * P]
                    nc.tensor.matmul(pb[:, 0:2 * ncols], lb, va[cidx][:],
                                     start=(cidx == 0), stop=(cidx == 3))
                zrow = wpool.tile([P, ROWW], BF16, tag="zrow")
                nc.scalar.copy(zrow[:, 0:D], pa[:])
                nc.scalar.copy(zrow[:, D:D + ncols], pb[:, 0:ncols])
                nc.vector.tensor_copy(ed_store[:, nt, :], pb[:, ncols:2 * ncols])
                nc.sync.dma_start(zloc[nt * P:(nt + 1) * P, :], zrow[:])

            # ---------- phase 1: z1 shard ----------
            xTb = [cpool.tile([P, SHN], BF16, tag=f"xT{c}", name=f"xT{c}") for c in range(4)]
            for cidx in range(4):
                xf = wpool.tile([P, SHN], F32, tag="xload")
                nc.sync.dma_start(xf[:], xT_in[cidx * P:(cidx + 1) * P, :])
                nc.vector.tensor_copy(xTb[cidx][:], xf[:])

            for nt in range(NT):
                z_tile(nt, [xTb[c][:] for c in range(4)], w1b, va1, zloc1,
                       ed1, H)

            nc.gpsimd.collective_compute(
                "AllGather", mybir.AluOpType.bypass,
                replica_groups=[list(range(NCORES))],
                ins=[zloc1[:]], outs=[ztab1[0:N, :]])

            # ztab2 is chunk-major: chunk ch = contiguous rows
            # [ch*NCORES*CHR, (ch+1)*NCORES*CHR), rank-major inside.
            CHR = SHN // NCH

            # ---------- layer-1 edge phase ----------
            # per-tile epilogue: elu -> bf16 -> transpose into hcTb,
            # then the layer-2 z row for this tile + chunked AllGather.
            def l1_out(nt, t1):
                # elu via ScalarE: em=relu(-t1); ex=exp(-em); pos=relu(t1)
                em = wpool.tile([P, D], F32, tag="em")
                nc.scalar.activation(em[:], t1[:],
                                     mybir.ActivationFunctionType.Relu,
                                     scale=-1.0)
                ex = wpool.tile([P, D], F32, tag="ex")
                nc.scalar.activation(ex[:], em[:],
                                     mybir.ActivationFunctionType.Exp,
                                     scale=-1.0)
                pos = wpool.tile([P, D], F32, tag="pos")
                nc.scalar.activation(pos[:], t1[:],
                                     mybir.ActivationFunctionType.Relu)
                hc = wpool.tile([P, D], BF16, tag="hc")
                nc.vector.scalar_tensor_tensor(
                    hc[:], ex[:], -1.0, pos[:],
                    mybir.AluOpType.add, mybir.AluOpType.add)
                for cidx in range(4):
                    pt = ptpool.tile([P, P], BF16, tag="ptr")
                    nc.tensor.transpose(pt[:], hc[:, cidx * P:(cidx + 1) * P],
                                        identb[:])
                    nc.vector.tensor_copy(
                        hcTb[cidx][:, nt * P:(nt + 1) * P], pt[:])
                # layer-2 z row for this tile, and chunked AllGather
                z_tile(nt, [hcTb[c][:] for c in range(4)], w2b, va2, zloc2,
                       ed2, 1)
                if nt % TPC == TPC - 1:
                    ch = nt // TPC
                    nc.gpsimd.collective_compute(
                        "AllGather", mybir.AluOpType.bypass,
                        replica_groups=[list(range(NCORES))],
                        ins=[zloc2[CHR * ch:CHR * (ch + 1), :]],
                        outs=[ztab2[NCORES * CHR * ch:
                                    NCORES * CHR * (ch + 1), :]])

            if PREP:
                gpsimd_gate(ztab1, [N - 1], "1")
            idx_off = 0
            for nt in range(NT):
                Kj = KT[nt]
                nkg = Kj // KG
                po = pepool.tile([P, D], F32, tag="pout")
                den = wpool.tile([P, H], F32, tag="den")
                for kg in range(nkg):
                    g = gpool.tile([P, KG, ROWW], BF16, tag="G")
                    nidx = P * KG
                    emit_gather(g, ztab1, idx1, idx_off, nidx)
                    idx_off += nidx // 16
                    # attention logits: s = es_gather + ed_local (dup'd pairs)
                    sd = wpool.tile([P, KG, H, 2], F32, tag="sd")
                    es_v = g[:, :, D:D + H].unsqueeze(3) \
                        .broadcast_to([P, KG, H, 2])
                    ed_v = ed1[:, nt, :].unsqueeze(1).unsqueeze(3) \
                        .broadcast_to([P, KG, H, 2])
                    nc.vector.tensor_tensor(sd[:], es_v, ed_v,
                                            mybir.AluOpType.add)
                    ud = wpool.tile([P, KG, H, 2], F32, tag="ud")
                    nc.vector.tensor_scalar_mul(ud[:], sd[:], ALPHA)
                    nc.vector.tensor_tensor(sd[:], sd[:], ud[:],
                                            mybir.AluOpType.max)
                    ad = wpool.tile([P, KG, H, 2], BF16, tag="ad")
                    nc.scalar.activation(ad[:], sd[:],
                                         mybir.ActivationFunctionType.Exp)
                    dpart = wpool.tile([P, H], F32, tag="dpart")
                    nc.vector.tensor_reduce(
                        dpart[:], ad[:].rearrange("p k h t -> p h k t"),
                        mybir.AxisListType.XY, mybir.AluOpType.add)
                    if kg == 0:
                        nc.vector.tensor_copy(den[:], dpart[:])
                    else:
                        nc.vector.tensor_tensor(den[:], den[:], dpart[:],
                                                mybir.AluOpType.add)
                    for k in range(KG):
                        wg = wgpool.tile([P, D], BF16, tag="wg")
                        g_v = g[:, k, 0:D].rearrange(
                            "p (h r t) -> p h r t", h=H, r=DO // 2, t=2)
                        a_v = ad[:, k, :, :].unsqueeze(2) \
                            .broadcast_to([P, H, DO // 2, 2])
                        w_v = wg[:].rearrange(
                            "p (h r t) -> p h r t", h=H, r=DO // 2, t=2)
                        nc.vector.tensor_tensor(w_v, g_v, a_v,
                                                mybir.AluOpType.mult)
                        kk = kg * KG + k
                        nc.tensor.matmul(po[:], identb[:], wg[:],
                                         start=(kk == 0), stop=(kk == Kj - 1))
                # normalize (x2 compensates the dup'd den) and activation
                rcp = wpool.tile([P, H], F32, tag="rcp")
                nc.vector.reciprocal(rcp[:], den[:])
                t1 = wpool.tile([P, D], F32, tag="t1")
                r_v = rcp[:].unsqueeze(2).broadcast_to([P, H, DO])
                t_v = t1[:].rearrange("p (h r) -> p h r", h=H, r=DO)
                nc.vector.scalar_tensor_tensor(
                    t_v, po[:].rearrange("p (h r) -> p h r", h=H, r=DO),
                    2.0, r_v, mybir.AluOpType.mult, mybir.AluOpType.mult)
                l1_out(nt, t1)

            # ---------- layer-2 edge phase: diag(exp) matmul accumulate -----
            if PREP:
                gpsimd_gate(ztab2, [NCORES * CHR * ch for ch in range(NCH)],
                            "2")
            idx_off = 0
            for nt in range(NT):
                Kj = KT[nt]
                nkg = Kj // KG
                po = pepool.tile([P, D], F32, tag="pout")
                den = wpool.tile([P, 1], F32, tag="den2")
                for kg in range(nkg):
                    g = gpool.tile([P, KG, ROWW], BF16, tag="G")
                    nidx = P * KG
                    emit_gather(g, ztab2, idx2, idx_off, nidx)
                    idx_off += nidx // 16
                    sd = wpool.tile([P, KG], F32, tag="sd2")
                    es_v = g[:, :, D:D + 1].squeeze(2)
                    ed_v = ed2[:, nt, :].broadcast_to([P, KG])
                    nc.vector.tensor_tensor(sd[:], es_v, ed_v,
                                            mybir.AluOpType.add)
                    ud = wpool.tile([P, KG], F32, tag="ud2")
                    nc.vector.tensor_scalar_mul(ud[:], sd[:], ALPHA)
                    nc.vector.tensor_tensor(sd[:], sd[:], ud[:],
                                            mybir.AluOpType.max)
                    ad = wpool.tile([P, KG], BF16, tag="ad2")
                    nc.scalar.activation(ad[:], sd[:],
                                         mybir.ActivationFunctionType.Exp)
                    dpart = wpool.tile([P, 1], F32, tag="dpart2")
                    nc.vector.tensor_reduce(
                        dpart[:], ad[:], mybir.AxisListType.X,
                        mybir.AluOpType.add)
                    if kg == 0:
                        nc.vector.tensor_copy(den[:], dpart[:])
                    else:
                        nc.vector.tensor_tensor(den[:], den[:], dpart[:],
                                                mybir.AluOpType.add)
                    # diag(exp) stationary: dg[p,k,j] = I[p,j] * ad[p,k]
                    dg = wgpool.tile([P, KG, P], BF16, tag="dg")
                    id_v = identb[:].unsqueeze(1).broadcast_to([P, KG, P])
                    ad_v = ad[:].unsqueeze(2).broadcast_to([P, KG, P])
                    nc.vector.tensor_tensor(dg[:], id_v, ad_v,
                                            mybir.AluOpType.mult)
                    for k in range(KG):
                        kk = kg * KG + k
                        nc.tensor.matmul(po[:], dg[:, k, :], g[:, k, 0:D],
                                         start=(kk == 0), stop=(kk == Kj - 1))
                # normalize (no dup here) + elu + residual
                rcp = wpool.tile([P, 1], F32, tag="rcp2")
                nc.vector.reciprocal(rcp[:], den[:])
                t1 = wpool.tile([P, D], F32, tag="t1")
                r_v = rcp[:].broadcast_to([P, D])
                nc.vector.scalar_tensor_tensor(
                    t1[:], po[:], 1.0, r_v,
                    mybir.AluOpType.mult, mybir.AluOpType.mult)
                em = wpool.tile([P, D], F32, tag="em")
                nc.scalar.activation(em[:], t1[:],
                                     mybir.ActivationFunctionType.Relu,
                                     scale=-1.0)
                ex = wpool.tile([P, D], F32, tag="ex")
                nc.scalar.activation(ex[:], em[:],
                                     mybir.ActivationFunctionType.Exp,
                                     scale=-1.0)
                pos = wpool.tile([P, D], F32, tag="pos")
                nc.scalar.activation(pos[:], t1[:],
                                     mybir.ActivationFunctionType.Relu)
                el = wpool.tile([P, D], F32, tag="el")
                nc.vector.scalar_tensor_tensor(
                    el[:], ex[:], -1.0, pos[:],
                    mybir.AluOpType.add, mybir.AluOpType.add)
                xr = wpool.tile([P, D], F32, tag="xr")
                nc.sync.dma_start(xr[:], x_in[nt * P:(nt + 1) * P, :])
                ot = wpool.tile([P, D], F32, tag="ot")
                nc.vector.tensor_tensor(ot[:], el[:], xr[:],
                                        mybir.AluOpType.add)
                nc.sync.dma_start(out[nt * P:(nt + 1) * P, :], ot[:])

    # Prep-mode (gen_mode=1) gathers: Tile pre-bumps the DMASW lane sems at
    # Pool-issue time (InstIncSwdgeSem), so consumer waits on those sems are
    # satisfied before the DMA lands.  The descriptor-baked sems (dma_sems,
    # +16 per gather at true DMA completion) are the only data-ready signal.
    # Remap every DMASW-lane wait onto the right queue sem + completion
    # count: lane l wait 16*t  ->  gather k=(t-1)*8+l, queue k%2, 16*(k//2+1).
    nlanes = 8
    for f in nc.m.functions:
        for bb in f.blocks:
            for ins in bb.instructions:
                si = ins.sync_info
                if not si:
                    continue
                for w in (si.on_wait or []):
                    nm = getattr(w, "ant_name", "") or ""
                    if nm.startswith("DMASW"):
                        lane = int(nm[5:].split("_")[0])
                        t = w.wait_value // 16
                        assert w.wait_value == 16 * t and t >= 1, (nm, w.wait_value)
                        k = (t - 1) * nlanes + lane
                        w.id = dma_sems[k % 2].num
                        w.wait_value = 16 * (k // 2 + 1)

    nc.compile()
    return nc


def kernel(h, W1, a1, Wout, aout, src, dst):
    h = np.asarray(h, np.float32)
    W1 = np.asarray(W1, np.float32)
    a1 = np.asarray(a1, np.float32)
    Wout = np.asarray(Wout, np.float32)
    aout = np.asarray(aout, np.float32)
    src = np.asarray(src, np.int32)
    dst = np.asarray(dst, np.int32)

    x = h.reshape(N, D)
    nodes, K_sched, totK, idx1_cores, idx2_cores = _build_host(src, dst)

    key = (tuple(int(k) for k in K_sched), totK)
    if key not in _cache:
        _cache[key] = _build_program(K_sched, totK)
    nc = _cache[key]

    # weight layouts
    W1cat = np.ascontiguousarray(W1.transpose(1, 0, 2).reshape(D, D))
    A1 = np.zeros((D, 16), np.float32)
    for hh in range(H):
        A1[hh * DO:(hh + 1) * DO, hh] = a1[hh, :DO]
        A1[hh * DO:(hh + 1) * DO, 8 + hh] = a1[hh, DO:]
    A2 = np.stack([aout[:D], aout[D:]], axis=1).astype(np.float32)
    ident = np.eye(P, dtype=np.float32)

    in_maps = []
    for c in range(NCORES):
        xs = np.ascontiguousarray(x[nodes[c]])
        in_maps.append({
            "xT": np.ascontiguousarray(xs.T),
            "x": xs,
            "w1": W1cat,
            "w1t": np.ascontiguousarray(W1cat.T),
            "a1": A1,
            "w2": Wout,
            "w2t": np.ascontiguousarray(Wout.T),
            "a2": A2,
            "ident": ident,
            "idx1": idx1_cores[c],
            "idx2": idx2_cores[c],
        })

    trace = bool(int(os.environ.get("GAT_TRACE", "0")))
    res = run_bass_kernel_spmd(nc, in_maps, core_ids=list(range(NCORES)),
                               trace=trace)
    if trace:
        print("HW exec time:", res.exec_time_ns, "ns")
        print("trace:", res.instructions_and_trace[1]
              if res.instructions_and_trace else None)
    outf = np.zeros((N, D), np.float32)
    for c in range(NCORES):
        outf[nodes[c]] = res.results[c]["out"]
    return outf.reshape(B, S, D)


# revision 35
# speedup vs baseline: 1.1850x; 1.1850x over previous
"""Trainium2 Bass kernel for a 2-layer multi-head GAT (nn_MultiHeadGATLayer).

Architecture recap (hardcoded, matches the reference):
  N=16384 nodes, D=512 feats, E=540672 edges (32 random in-edges/node + self loop),
  layer 1: 8 heads x 64 dims with per-head attention + elu, concat;
  layer 2: single 512-dim GAT head over the concat + elu; residual with input.

Distribution: nodes are sharded across 8 NeuronCores (destination sharding).
Each core computes z = x @ W for its node shard, the shards are AllGathered
into a per-core z-table in HBM, and each core then runs the edge phase for its
own destination nodes.

Performance structure (v2):
  - Gathers are SWDGE prepare_only + trigger_dma on 2 alternating queues so
    descriptor generation (GPSIMD) decouples from the DMA transfer; gather
    buffers are 6-deep so transfers pipeline back-to-back.
  - Layer-2 z rows are computed per-tile inside the layer-1 edge loop and
    AllGathered in 4 chunks that overlap the layer-1 edge phase.
  - Layer-2 edge accumulate uses diag(exp) as the matmul stationary operand
    (one [P,KG,128] diag build per group) instead of per-slot DVE multiplies.
  - PSUM->SBUF z-row copies run on the Scalar engine.

Host side does layout only: degree-sorted node permutation, padded gather
index construction (int16, 16-partition wrap), weight reshapes/transposes.
"""
import os
import sys

sys.path.insert(0, "/opt/trn_rl_repo")

import numpy as np
import ml_dtypes

import concourse.bacc as bacc
import concourse.mybir as mybir
from concourse.tile import TileContext
from concourse.bass_utils import run_bass_kernel_spmd
from concourse.library_config import mlp

F32 = mybir.dt.float32
BF16 = mybir.dt.bfloat16
I16 = mybir.dt.int16

B, S, D = 64, 256, 512
H, DO = 8, 64
ALPHA = 0.2
N = B * S
DEG = 32
E = N * DEG + N
NCORES = 8
P = 128
SHN = N // NCORES          # nodes per core (2048)
NT = SHN // P              # node tiles per core (16)
KG = 8                     # slots per gather call
ROWW = 640                 # z-table row width (512 z + 8 es + pad), bf16
DUMMY = N                  # dummy row index for padding slots
NEG = -3.0e38
NCH = 4                    # zloc2 AllGather chunks
TPC = NT // NCH            # tiles per chunk

_cache = {}


def _build_host(src, dst):
    """Host-side layout: permutation, per-core padded gather indices.

    ztab1 is laid out core-major (row = core*SHN + pos, the natural
    one-shot AllGather order).  ztab2 is laid out chunk-major
    (row = chunk*(NCORES*CHR) + core*CHR + pos%CHR, CHR = SHN//NCH) so each
    of the NCH chunked AllGathers writes a contiguous row range.
    """
    deg = np.bincount(dst, minlength=N)
    order = np.argsort(-deg, kind="stable")          # nodes by degree desc
    # deal round-robin so all cores see the same degree profile
    core_of = np.empty(N, np.int32)
    pos_of = np.empty(N, np.int32)
    for c in range(NCORES):
        nodes_c = order[c::NCORES]                   # 2048 nodes, deg-sorted desc
        core_of[nodes_c] = c
        pos_of[nodes_c] = np.arange(SHN)
    nodes = [order[c::NCORES] for c in range(NCORES)]
    core64 = core_of.astype(np.int64)
    pos64 = pos_of.astype(np.int64)
    tabpos1 = (core64 * SHN + pos64).astype(np.int32)
    CHR = SHN // NCH
    tabpos2 = ((pos64 // CHR) * (NCORES * CHR) + core64 * CHR
               + (pos64 % CHR)).astype(np.int32)

    # in-edges per node: sort edges by dst
    eorder = np.argsort(dst, kind="stable")
    src_s = src[eorder]
    cum = np.zeros(N + 1, np.int64)
    np.cumsum(deg, out=cum[1:])

    # common slot schedule: K_sched[j] = max over cores of tile max degree
    K_sched = np.zeros(NT, np.int64)
    for c in range(NCORES):
        dg = deg[nodes[c]]
        for j in range(NT):
            K_sched[j] = max(K_sched[j], dg[j * P:(j + 1) * P].max())
    K_sched = ((K_sched + KG - 1) // KG) * KG
    totK = int(K_sched.sum())

    # padded source table per core: [NT, K_j, P] slot-major within tile
    def build_idx(tabpos):
        idx_cores = []
        for c in range(NCORES):
            blocks = []
            nds = nodes[c]
            for j in range(NT):
                nj = nds[j * P:(j + 1) * P]
                Kj = int(K_sched[j])
                pad = np.full((P, Kj), DUMMY, np.int32)
                for i, n in enumerate(nj):
                    d0 = int(deg[n])
                    pad[i, :d0] = tabpos[src_s[cum[n]:cum[n] + d0]]
                blocks.append(pad.T.reshape(-1))     # slot-major: [Kj, P]
            flat = np.concatenate(blocks).astype(np.int32)   # [totK*P]
            # int16 wrap: idx i -> partition i%16, col i//16 ; replicate 8x
            assert flat.max() <= 32767
            w = flat.reshape(-1, 16).T               # [16, totK*P/16]
            idx_cores.append(np.tile(w, (8, 1)).astype(np.int16))
        return idx_cores

    return nodes, K_sched, totK, build_idx(tabpos1), build_idx(tabpos2)


def _build_program(K_sched, totK):
    prep_mode = bool(int(os.environ.get("GAT_PREP", "0")))
    nc = bacc.Bacc("TRN2", target_bir_lowering=False, debug=False,
                   num_devices=NCORES, num_swdge_queues=2)
    KT = [int(k) for k in K_sched]
    IDXW = totK * P // 16

    xT_in = nc.dram_tensor("xT", [D, SHN], F32, kind="ExternalInput")
    x_in = nc.dram_tensor("x", [SHN, D], F32, kind="ExternalInput")
    w1_in = nc.dram_tensor("w1", [D, D], F32, kind="ExternalInput")       # W1cat
    w1t_in = nc.dram_tensor("w1t", [D, D], F32, kind="ExternalInput")     # W1cat.T
    a1_in = nc.dram_tensor("a1", [D, 16], F32, kind="ExternalInput")      # [A1s | A1d]
    w2_in = nc.dram_tensor("w2", [D, D], F32, kind="ExternalInput")       # Wout
    w2t_in = nc.dram_tensor("w2t", [D, D], F32, kind="ExternalInput")     # Wout.T
    a2_in = nc.dram_tensor("a2", [D, 2], F32, kind="ExternalInput")       # [aout_s | aout_d]
    id_in = nc.dram_tensor("ident", [P, P], F32, kind="ExternalInput")
    idx1_in = nc.dram_tensor("idx1", [P, IDXW], I16, kind="ExternalInput")
    idx2_in = nc.dram_tensor("idx2", [P, IDXW], I16, kind="ExternalInput")
    out = nc.dram_tensor("out", [SHN, D], F32, kind="ExternalOutput")

    zloc1 = nc.dram_tensor("zloc1", [SHN, ROWW], BF16, kind="Internal")
    zloc2 = nc.dram_tensor("zloc2", [SHN, ROWW], BF16, kind="Internal")
    ztab1 = nc.dram_tensor("ztab1", [N + P, ROWW], BF16, kind="Internal",
                           addr_space="Shared")
    ztab2 = nc.dram_tensor("ztab2", [N + P, ROWW], BF16, kind="Internal",
                           addr_space="Shared")

    with TileContext(nc) as tc:
        with tc.tile_pool(name="const", bufs=1) as cpool, \
             tc.tile_pool(name="work", bufs=2) as wpool, \
             tc.tile_pool(name="gat", bufs=6) as gpool, \
             tc.tile_pool(name="wg", bufs=3) as wgpool, \
             tc.tile_pool(name="pz", bufs=2, space="PSUM") as pzpool, \
             tc.tile_pool(name="pe", bufs=2, space="PSUM") as pepool, \
             tc.tile_pool(name="pt", bufs=2, space="PSUM") as ptpool:

            nc.gpsimd.load_library(mlp)

            dma_sems = [nc.alloc_semaphore(f"gdma{q}") for q in range(2)]
            for s in dma_sems:
                nc.gpsimd.sem_clear(s)

            # In prep mode Tile drops the collective->gather RAW dep (it
            # should defer to the trigger but doesn't), so gate the in-order
            # GPSIMD stream: DMA-read a row the collective wrote, then have
            # GPSIMD consume it before the first prep of the edge phase.
            def gpsimd_gate(ztab, row_ids, tag):
                chk = cpool.tile([1, len(row_ids), 2], BF16, tag=f"chk{tag}")
                for j, r in enumerate(row_ids):
                    nc.sync.dma_start(chk[:, j, :], ztab[r:r + 1, 0:2])
                gate = cpool.tile([1, 2], BF16, tag=f"gate{tag}")
                for j in range(len(row_ids)):
                    nc.gpsimd.tensor_copy(gate[:], chk[:, j, :])

            # ---------- setup: constants, weights ----------
            identf = cpool.tile([P, P], F32)
            identb = cpool.tile([P, P], BF16)
            nc.sync.dma_start(identf[:], id_in[:])
            nc.vector.tensor_copy(identb[:], identf[:])

            idx1 = cpool.tile([P, IDXW], I16)
            idx2 = cpool.tile([P, IDXW], I16)
            nc.sync.dma_start(idx1[:], idx1_in[:])
            nc.sync.dma_start(idx2[:], idx2_in[:])

            # weights: rhs chunks [128, 512] bf16 (4 per layer) + va cols
            w1b, w2b = [], []
            for cidx in range(4):
                wf = wpool.tile([P, D], F32, tag="wload")
                nc.sync.dma_start(wf[:], w1_in[cidx * P:(cidx + 1) * P, :])
                wb = cpool.tile([P, D], BF16, tag=f"w1b{cidx}")
                nc.vector.tensor_copy(wb[:], wf[:])
                w1b.append(wb)
                wf2 = wpool.tile([P, D], F32, tag="wload")
                nc.sync.dma_start(wf2[:], w2_in[cidx * P:(cidx + 1) * P, :])
                wb2 = cpool.tile([P, D], BF16, tag=f"w2b{cidx}")
                nc.vector.tensor_copy(wb2[:], wf2[:])
                w2b.append(wb2)

            a1f = [cpool.tile([P, 16], F32, tag=f"a1f{c}", name=f"a1f{c}") for c in range(4)]
            a2f = [cpool.tile([P, 2], F32, tag=f"a2f{c}", name=f"a2f{c}") for c in range(4)]
            for cidx in range(4):
                nc.sync.dma_start(a1f[cidx][:], a1_in[cidx * P:(cidx + 1) * P, :])
                nc.sync.dma_start(a2f[cidx][:], a2_in[cidx * P:(cidx + 1) * P, :])

            # va1[d,16] = W1cat.T-chunks.T @ A1 blocks ; va2[d,2].
            # w1t/w2t chunks stream through one 4-deep wpool tag ring.
            va1 = []
            va2 = []
            wtf = []
            for cidx in range(4):
                t = wpool.tile([P, D], F32, tag="wt", bufs=4)
                nc.sync.dma_start(t[:], w1t_in[cidx * P:(cidx + 1) * P, :])
                wtf.append(t)
            for dchunk in range(4):
                pv = pzpool.tile([P, 16], F32, tag="pzA")
                for fc in range(4):
                    nc.tensor.matmul(
                        pv[:], wtf[fc][:, dchunk * P:(dchunk + 1) * P],
                        a1f[fc][:],
                        start=(fc == 0), stop=(fc == 3))
                vb = cpool.tile([P, 16], BF16, tag=f"va1{dchunk}")
                nc.vector.tensor_copy(vb[:], pv[:])
                va1.append(vb)
            wtf = []
            for cidx in range(4):
                t = wpool.tile([P, D], F32, tag="wt", bufs=4)
                nc.sync.dma_start(t[:], w2t_in[cidx * P:(cidx + 1) * P, :])
                wtf.append(t)
            for dchunk in range(4):
                pv2 = pzpool.tile([P, 2], F32, tag="pzB", bufs=1)
                for fc in range(4):
                    nc.tensor.matmul(
                        pv2[:], wtf[fc][:, dchunk * P:(dchunk + 1) * P],
                        a2f[fc][:],
                        start=(fc == 0), stop=(fc == 3))
                vb2 = cpool.tile([P, 2], BF16, tag=f"va2{dchunk}")
                nc.vector.tensor_copy(vb2[:], pv2[:])
                va2.append(vb2)

            # dummy row (padding target): zeros except es cols = NEG
            drow = cpool.tile([1, ROWW], BF16)
            nc.vector.memset(drow[:], 0.0)
            nc.vector.memset(drow[:, 512:520], NEG)
            for ztab in (ztab1, ztab2):
                nc.sync.dma_start(ztab[N:N + 1, :], drow[:])

            ed1 = cpool.tile([P, NT, H], F32)
            ed2 = cpool.tile([P, NT, 1], F32)
            hcTb = [cpool.tile([P, SHN], BF16, tag=f"hcT{c}", name=f"hcT{c}") for c in range(4)]

            # ---------- gather emitter: prep + trigger on alternating queues --
            qtog = [0]

            PREP = prep_mode

            def emit_gather(g, ztab, idx, idx_off, nidx):
                if not PREP:
                    nc.gpsimd.dma_gather(
                        g[:], ztab[:], idx[:, idx_off:idx_off + nidx // 16],
                        nidx, nidx, ROWW)
                    return
                q = qtog[0]
                qtog[0] = 1 - q
                nc.gpsimd.dma_gather(
                    g[:], ztab[:], idx[:, idx_off:idx_off + nidx // 16],
                    nidx, nidx, ROWW,
                    prepare_only=True, sem=dma_sems[q], queue_num=q)
                nc.gpsimd.trigger_dma(count=None, queue_num=q)

            # ---------- z tile: z row + attn cols for one 128-node tile ------
            def z_tile(nt, lhs_blocks, wb, va, zloc, ed_store, ncols):
                pa = pzpool.tile([P, D], F32, tag="pzA")
                pb = pzpool.tile([P, 16], F32, tag="pzB", bufs=1)
                for cidx in range(4):
                    lb = lhs_blocks[cidx][:, nt * P:(nt + 1) * P]
                    nc.tensor.matmul(pa[:], lb, wb[cidx][:],
                                     start=(cidx == 0), stop=(cidx == 3))
                for cidx in range(4):
                    lb = lhs_blocks[cidx][:, nt * P:(nt + 1) * P]
                    nc.tensor.matmul(pb[:, 0:2 * ncols], lb, va[cidx][:],
                                     start=(cidx == 0), stop=(cidx == 3))
                zrow = wpool.tile([P, ROWW], BF16, tag="zrow")
                nc.scalar.copy(zrow[:, 0:D], pa[:])
                nc.scalar.copy(zrow[:, D:D + ncols], pb[:, 0:ncols])
                nc.vector.tensor_copy(ed_store[:, nt, :], pb[:, ncols:2 * ncols])
                nc.sync.dma_start(zloc[nt * P:(nt + 1) * P, :], zrow[:])

            # ---------- phase 1: z1 shard ----------
            xTb = [cpool.tile([P, SHN], BF16, tag=f"xT{c}", name=f"xT{c}") for c in range(4)]
            for cidx in range(4):
                xf = wpool.tile([P, SHN], F32, tag="xload")
                nc.sync.dma_start(xf[:], xT_in[cidx * P:(cidx + 1) * P, :])
                nc.vector.tensor_copy(xTb[cidx][:], xf[:])

            for nt in range(NT):
                z_tile(nt, [xTb[c][:] for c in range(4)], w1b, va1, zloc1,
                       ed1, H)

            nc.gpsimd.collective_compute(
                "AllGather", mybir.AluOpType.bypass,
                replica_groups=[list(range(NCORES))],
                ins=[zloc1[:]], outs=[ztab1[0:N, :]])

            # ztab2 is chunk-major: chunk ch = contiguous rows
            # [ch*NCORES*CHR, (ch+1)*NCORES*CHR), rank-major inside.
            CHR = SHN // NCH

            # ---------- layer-1 edge phase ----------
            # per-tile epilogue: elu -> bf16 -> transpose into hcTb,
            # then the layer-2 z row for this tile + chunked AllGather.
            def l1_out(nt, t1):
                # elu via ScalarE: em=relu(-t1); ex=exp(-em); pos=relu(t1)
                em = wpool.tile([P, D], F32, tag="em")
                nc.scalar.activation(em[:], t1[:],
                                     mybir.ActivationFunctionType.Relu,
                                     scale=-1.0)
                ex = wpool.tile([P, D], F32, tag="ex")
                nc.scalar.activation(ex[:], em[:],
                                     mybir.ActivationFunctionType.Exp,
                                     scale=-1.0)
                pos = wpool.tile([P, D], F32, tag="pos")
                nc.scalar.activation(pos[:], t1[:],
                                     mybir.ActivationFunctionType.Relu)
                hc = wpool.tile([P, D], BF16, tag="hc")
                nc.vector.scalar_tensor_tensor(
                    hc[:], ex[:], -1.0, pos[:],
                    mybir.AluOpType.add, mybir.AluOpType.add)
                for cidx in range(4):
                    pt = ptpool.tile([P, P], BF16, tag="ptr")
                    nc.tensor.transpose(pt[:], hc[:, cidx * P:(cidx + 1) * P],
                                        identb[:])
                    nc.vector.tensor_copy(
                        hcTb[cidx][:, nt * P:(nt + 1) * P], pt[:])
                # layer-2 z row for this tile, and chunked AllGather
                z_tile(nt, [hcTb[c][:] for c in range(4)], w2b, va2, zloc2,
                       ed2, 1)
                if nt % TPC == TPC - 1:
                    ch = nt // TPC
                    nc.gpsimd.collective_compute(
                        "AllGather", mybir.AluOpType.bypass,
                        replica_groups=[list(range(NCORES))],
                        ins=[zloc2[CHR * ch:CHR * (ch + 1), :]],
                        outs=[ztab2[NCORES * CHR * ch:
                                    NCORES * CHR * (ch + 1), :]])

            if PREP:
                gpsimd_gate(ztab1, [N - 1], "1")
            idx_off = 0
            for nt in range(NT):
                Kj = KT[nt]
                nkg = Kj // KG
                po = pepool.tile([P, D], F32, tag="pout")
                den = wpool.tile([P, H], F32, tag="den")
                for kg in range(nkg):
                    g = gpool.tile([P, KG, ROWW], BF16, tag="G")
                    nidx = P * KG
                    emit_gather(g, ztab1, idx1, idx_off, nidx)
                    idx_off += nidx // 16
                    # attention logits: s = es_gather + ed_local (dup'd pairs)
                    sd = wpool.tile([P, KG, H, 2], F32, tag="sd")
                    es_v = g[:, :, D:D + H].unsqueeze(3) \
                        .broadcast_to([P, KG, H, 2])
                    ed_v = ed1[:, nt, :].unsqueeze(1).unsqueeze(3) \
                        .broadcast_to([P, KG, H, 2])
                    nc.vector.tensor_tensor(sd[:], es_v, ed_v,
                                            mybir.AluOpType.add)
                    ud = wpool.tile([P, KG, H, 2], F32, tag="ud")
                    nc.vector.tensor_scalar_mul(ud[:], sd[:], ALPHA)
                    nc.vector.tensor_tensor(sd[:], sd[:], ud[:],
                                            mybir.AluOpType.max)
                    ad = wpool.tile([P, KG, H, 2], BF16, tag="ad")
                    nc.scalar.activation(ad[:], sd[:],
                                         mybir.ActivationFunctionType.Exp)
                    dpart = wpool.tile([P, H], F32, tag="dpart")
                    nc.vector.tensor_reduce(
                        dpart[:], ad[:].rearrange("p k h t -> p h k t"),
                        mybir.AxisListType.XY, mybir.AluOpType.add)
                    if kg == 0:
                        nc.vector.tensor_copy(den[:], dpart[:])
                    else:
                        nc.vector.tensor_tensor(den[:], den[:], dpart[:],
                                                mybir.AluOpType.add)
                    for k in range(KG):
                        wg = wgpool.tile([P, D], BF16, tag="wg")
                        g_v = g[:, k, 0:D].rearrange(
                            "p (h r t) -> p h r t", h=H, r=DO // 2, t=2)
                        a_v = ad[:, k, :, :].unsqueeze(2) \
                            .broadcast_to([P, H, DO // 2, 2])
                        w_v = wg[:].rearrange(
                            "p (h r t) -> p h r t", h=H, r=DO // 2, t=2)
                        nc.vector.tensor_tensor(w_v, g_v, a_v,
                                                mybir.AluOpType.mult)
                        kk = kg * KG + k
                        nc.tensor.matmul(po[:], identb[:], wg[:],
                                         start=(kk == 0), stop=(kk == Kj - 1))
                # normalize (x2 compensates the dup'd den) and activation
                rcp = wpool.tile([P, H], F32, tag="rcp")
                nc.vector.reciprocal(rcp[:], den[:])
                t1 = wpool.tile([P, D], F32, tag="t1")
                r_v = rcp[:].unsqueeze(2).broadcast_to([P, H, DO])
                t_v = t1[:].rearrange("p (h r) -> p h r", h=H, r=DO)
                nc.vector.scalar_tensor_tensor(
                    t_v, po[:].rearrange("p (h r) -> p h r", h=H, r=DO),
                    2.0, r_v, mybir.AluOpType.mult, mybir.AluOpType.mult)
                l1_out(nt, t1)

            # ---------- layer-2 edge phase: diag(exp) matmul accumulate -----
            if PREP:
                gpsimd_gate(ztab2, [NCORES * CHR * ch for ch in range(NCH)],
                            "2")
            idx_off = 0
            for nt in range(NT):
                Kj = KT[nt]
                nkg = Kj // KG
                po = pepool.tile([P, D], F32, tag="pout")
                den = wpool.tile([P, 1], F32, tag="den2")
                for kg in range(nkg):
                    g = gpool.tile([P, KG, ROWW], BF16, tag="G")
                    nidx = P * KG
                    emit_gather(g, ztab2, idx2, idx_off, nidx)
                    idx_off += nidx // 16
                    sd = wpool.tile([P, KG], F32, tag="sd2")
                    es_v = g[:, :, D:D + 1].squeeze(2)
                    ed_v = ed2[:, nt, :].broadcast_to([P, KG])
                    nc.vector.tensor_tensor(sd[:], es_v, ed_v,
                                            mybir.AluOpType.add)
                    ud = wpool.tile([P, KG], F32, tag="ud2")
                    nc.vector.tensor_scalar_mul(ud[:], sd[:], ALPHA)
                    nc.vector.tensor_tensor(sd[:], sd[:], ud[:],
                                            mybir.AluOpType.max)
                    ad = wpool.tile([P, KG], BF16, tag="ad2")
                    nc.scalar.activation(ad[:], sd[:],
                                         mybir.ActivationFunctionType.Exp)
                    dpart = wpool.tile([P, 1], F32, tag="dpart2")
                    nc.vector.tensor_reduce(
                        dpart[:], ad[:], mybir.AxisListType.X,
                        mybir.AluOpType.add)
                    if kg == 0:
                        nc.vector.tensor_copy(den[:], dpart[:])
                    else:
                        nc.vector.tensor_tensor(den[:], den[:], dpart[:],
                                                mybir.AluOpType.add)
                    # diag(exp) stationary: dg[p,k,j] = I[p,j] * ad[p,k]
                    dg = wgpool.tile([P, KG, P], BF16, tag="dg")
                    id_v = identb[:].unsqueeze(1).broadcast_to([P, KG, P])
                    ad_v = ad[:].unsqueeze(2).broadcast_to([P, KG, P])
                    nc.vector.tensor_tensor(dg[:], id_v, ad_v,
                                            mybir.AluOpType.mult)
                    for k in range(KG):
                        kk = kg * KG + k
                        nc.tensor.matmul(po[:], dg[:, k, :], g[:, k, 0:D],
                                         start=(kk == 0), stop=(kk == Kj - 1))
                # normalize (no dup here) + elu + residual
                rcp = wpool.tile([P, 1], F32, tag="rcp2")
                nc.vector.reciprocal(rcp[:], den[:])
                t1 = wpool.tile([P, D], F32, tag="t1")
                r_v = rcp[:].broadcast_to([P, D])
                nc.vector.scalar_tensor_tensor(
                    t1[:], po[:], 1.0, r_v,
                    mybir.AluOpType.mult, mybir.AluOpType.mult)
                em = wpool.tile([P, D], F32, tag="em")
                nc.scalar.activation(em[:], t1[:],
                                     mybir.ActivationFunctionType.Relu,
                                     scale=-1.0)
                ex = wpool.tile([P, D], F32, tag="ex")
                nc.scalar.activation(ex[:], em[:],
                                     mybir.ActivationFunctionType.Exp,
                                     scale=-1.0)
                pos = wpool.tile([P, D], F32, tag="pos")
                nc.scalar.activation(pos[:], t1[:],
                                     mybir.ActivationFunctionType.Relu)
                el = wpool.tile([P, D], F32, tag="el")
                nc.vector.scalar_tensor_tensor(
                    el[:], ex[:], -1.0, pos[:],
                    mybir.AluOpType.add, mybir.AluOpType.add)
                xr = wpool.tile([P, D], F32, tag="xr")
                nc.sync.dma_start(xr[:], x_in[nt * P:(nt + 1) * P, :])
                ot = wpool.tile([P, D], F32, tag="ot")
                nc.vector.tensor_tensor(ot[:], el[:], xr[:],
                                        mybir.AluOpType.add)
                nc.sync.dma_start(out[nt * P:(nt + 1) * P, :], ot[:])

    # Prep-mode (gen_mode=1) gathers: Tile pre-bumps the DMASW lane sems at
    # Pool-issue time (InstIncSwdgeSem), so consumer waits on those sems are
    # satisfied before the DMA lands.  The descriptor-baked sems (dma_sems,
    # +16 per gather at true DMA completion) are the only data-ready signal.
    # Remap every DMASW-lane wait onto the right queue sem + completion
    # count: lane l wait 16*t  ->  gather k=(t-1)*8+l, queue k%2, 16*(k//2+1).
    # (Plain-gather mode keeps Tile's own lane-sem protocol — no remap.)
    nlanes = 8
    for f in (nc.m.functions if prep_mode else []):
        for bb in f.blocks:
            for ins in bb.instructions:
                si = ins.sync_info
                if not si:
                    continue
                for w in (si.on_wait or []):
                    nm = getattr(w, "ant_name", "") or ""
                    if nm.startswith("DMASW"):
                        lane = int(nm[5:].split("_")[0])
                        t = w.wait_value // 16
                        assert w.wait_value == 16 * t and t >= 1, (nm, w.wait_value)
                        k = (t - 1) * nlanes + lane
                        w.id = dma_sems[k % 2].num
                        w.wait_value = 16 * (k // 2 + 1)

    nc.compile()
    return nc


def kernel(h, W1, a1, Wout, aout, src, dst):
    h = np.asarray(h, np.float32)
    W1 = np.asarray(W1, np.float32)
    a1 = np.asarray(a1, np.float32)
    Wout = np.asarray(Wout, np.float32)
    aout = np.asarray(aout, np.float32)
    src = np.asarray(src, np.int32)
    dst = np.asarray(dst, np.int32)

    x = h.reshape(N, D)
    nodes, K_sched, totK, idx1_cores, idx2_cores = _build_host(src, dst)

    key = (tuple(int(k) for k in K_sched), totK)
    if key not in _cache:
        _cache[key] = _build_program(K_sched, totK)
    nc = _cache[key]

    # weight layouts
    W1cat = np.ascontiguousarray(W1.transpose(1, 0, 2).reshape(D, D))
    A1 = np.zeros((D, 16), np.float32)
    for hh in range(H):
        A1[hh * DO:(hh + 1) * DO, hh] = a1[hh, :DO]
        A1[hh * DO:(hh + 1) * DO, 8 + hh] = a1[hh, DO:]
    A2 = np.stack([aout[:D], aout[D:]], axis=1).astype(np.float32)
    ident = np.eye(P, dtype=np.float32)

    in_maps = []
    for c in range(NCORES):
        xs = np.ascontiguousarray(x[nodes[c]])
        in_maps.append({
            "xT": np.ascontiguousarray(xs.T),
            "x": xs,
            "w1": W1cat,
            "w1t": np.ascontiguousarray(W1cat.T),
            "a1": A1,
            "w2": Wout,
            "w2t": np.ascontiguousarray(Wout.T),
            "a2": A2,
            "ident": ident,
            "idx1": idx1_cores[c],
            "idx2": idx2_cores[c],
        })

    trace = bool(int(os.environ.get("GAT_TRACE", "0")))
    res = run_bass_kernel_spmd(nc, in_maps, core_ids=list(range(NCORES)),
                               trace=trace)
    if trace:
        print("HW exec time:", res.exec_time_ns, "ns")
        print("trace:", res.instructions_and_trace[1]
              if res.instructions_and_trace else None)
    outf = np.zeros((N, D), np.float32)
    for c in range(NCORES):
        outf[nodes[c]] = res.results[c]["out"]
    return outf.reshape(B, S, D)
